# revision 47
# baseline (speedup 1.0000x reference)
"""Trainium2 Bass kernel for nn_GCN_5403068858882 (GCN + 3x GENConv + pool head).

Self-contained: schedule builder + bass program builder + SPMD runner.
See module docstring in the repo history for the design; in short:

- 8 cores, core c owns graphs [32c,32c+32) (contiguous nodes, batch sorted).
- Nodes packed into 32-slot bins (cap 3*128 "A" edges / 3*128 "B" edges,
  A = src graph < G/2 so dma_gather int16 indices fit).
- Per layer: node-space LN/PReLU -> table rows [A|B]=[exp(v), v*exp(v)]
  (bf16) -> AllGather -> per 128-edge tile: dma_gather rows + PE matmul with
  an is_equal selection matrix accumulating softmax numerator/denominator in
  PSUM -> agg=w/s+u -> MLP (bn folded) -> residual ledger.
- GCN conv: same machinery, f32 table h0*dinv, self loop via own-row add.
- Pooling: bf16 SBUF-source dma_gather (transpose) into a per-graph padded
  channel-major grid -> one reduce per stat -> tiny AllGather -> MLP head.
"""

import numpy as np
import ml_dtypes

import concourse.bass as bass
import concourse.bacc as bacc


import concourse.mybir as mybir
import concourse.tile as tile
from concourse.bass_utils import run_bass_kernel_spmd
from concourse._compat import get_trn_type

F32 = mybir.dt.float32
BF16 = mybir.dt.bfloat16
I16 = mybir.dt.int16
AF = mybir.ActivationFunctionType
ALU = mybir.AluOpType
NPBF = ml_dtypes.bfloat16

H = 64
F_IN = 5
L = 3
EPS_BN = 1e-5
EPS_MSG = 1e-7
NCORES = 8
TA = 3
TB = 3
BINCAP = 32
CHUNK_BINS = 8           # bins per gather superchunk
MOCK_COLLECTIVES = False  # replace AllGathers with local DMA (TimelineSim)
PHASES = 3               # debug: 1=conv only, 2=+GEN layers, 3=+pool/head
CONV_AG = True           # debug: run the conv AllGather
CONV_EDGE = True         # debug: run the conv edge phase
EDGE_GATHER = True       # debug: issue dma_gather calls
EDGE_MM = True           # debug: issue edge matmuls
# Per-call SWDGE descriptor cap: ucode dge_n_inflight=128 per direction;
# a gather call generates ~num_idxs/16+1 descs per direction (transpose
# gathers ~num_idxs/8 s2m), so edge calls must stay <=2032 idxs and pool
# transpose calls <=~1000 idxs or the device hangs.
GATHER_SPLIT = 3         # 1024-idx edge calls -> 65 descs/dir
DMA_SCRATCH = 16384      # default SWDGE ring (plenty: ring slots = ndesc)


# ---------------------------------------------------------------- schedule
class Sched:
    pass


def build_schedule(edge_index, batch_idx, G):
    s = Sched()
    src = np.asarray(edge_index[0], np.int64)
    dst = np.asarray(edge_index[1], np.int64)
    batch = np.asarray(batch_idx, np.int64)
    n = batch.shape[0]
    s.G = G
    s.GPC = GPC = G // NCORES

    deg = np.bincount(dst, minlength=n).astype(np.float64) + 1.0
    s.dinv_node = (deg ** -0.5).astype(np.float32)

    gstart = np.searchsorted(batch, np.arange(G))
    gend = np.searchsorted(batch, np.arange(G), side="right")
    s.cnt = cnt = gend - gstart

    # Balance graphs across cores by dst-edge count (LPT): core c's 32 graphs
    # are core_graphs[c]; the host unpermutes output rows via graph_order.
    gedge = np.bincount(batch[dst], minlength=G)
    load = np.zeros(NCORES, np.int64)
    slots_left = np.full(NCORES, GPC, np.int64)
    core_graphs = [[] for _ in range(NCORES)]
    for g in np.argsort(-gedge, kind="stable"):
        ok = np.flatnonzero(slots_left > 0)
        c = int(ok[np.argmin(load[ok])])
        core_graphs[c].append(int(g))
        load[c] += gedge[g]
        slots_left[c] -= 1
    core_of_graph = np.empty(G, np.int64)
    for c in range(NCORES):
        for g in core_graphs[c]:
            core_of_graph[g] = c
    s.core_graphs = core_graphs
    s.graph_order = np.concatenate(
        [np.asarray(cg, np.int64) for cg in core_graphs]
    )

    a_edge = core_of_graph[batch[src]] < (NCORES // 2)
    acnt = np.bincount(dst[a_edge], minlength=n)
    bcnt = np.bincount(dst[~a_edge], minlength=n)

    # Balanced (LPT/worst-fit) packing: nodes within a core may occupy any bin
    # (pool gathers look slots up per-graph). Partition a core's nodes into nb
    # bins of <=BINCAP slots / <=CAP_A A-edges / <=CAP_B B-edges, descending
    # by load, each into the fitting bin with most remaining capacity.
    CAP_A, CAP_B = TA * 128, TB * 128

    def _pack_lpt(nds, nb, key):
        order_n = nds[np.argsort(-key, kind="stable")]
        rem_a = np.full(nb, CAP_A, np.int64)
        rem_b = np.full(nb, CAP_B, np.int64)
        rem_s = np.full(nb, BINCAP, np.int64)
        rem_s[0] -= 2                     # two reserved invalid slots in bin 0
        bins = [[] for _ in range(nb)]
        bins[0] = [-1, -1]
        for nd in order_n:
            a, b = acnt[nd], bcnt[nd]
            ok = np.flatnonzero((rem_a >= a) & (rem_b >= b) & (rem_s >= 1))
            if len(ok) == 0:
                return None
            bi = int(ok[np.argmax(rem_a[ok] + rem_b[ok])])
            bins[bi].append(int(nd))
            rem_a[bi] -= a
            rem_b[bi] -= b
            rem_s[bi] -= 1
        return bins

    core_bins = []
    for c in range(NCORES):
        nds = np.concatenate(
            [np.arange(gstart[g], gend[g]) for g in core_graphs[c]]
        )
        nb = max(
            -(-(len(nds) + 2) // BINCAP),
            -(-int(acnt[nds].sum()) // CAP_A),
            -(-int(bcnt[nds].sum()) // CAP_B),
        )
        keys = (
            acnt[nds] + bcnt[nds],
            np.maximum(acnt[nds], bcnt[nds]),
            2 * acnt[nds] + bcnt[nds],
            acnt[nds] + 2 * bcnt[nds],
        )
        bins = None
        while bins is None:
            for key in keys:
                bins = _pack_lpt(nds, nb, key)
                if bins is not None:
                    break
            else:
                nb += 1
        core_bins.append(bins)

    NB = max(len(b) for b in core_bins)
    NB = -(-NB // CHUNK_BINS) * CHUNK_BINS
    s.NB = NB
    s.NSLOT = NSLOT = NB * BINCAP
    s.NBLK = NB // 4
    assert 4 * NSLOT <= 32768, NSLOT

    slot2node = np.full((NCORES, NSLOT), -1, np.int64)
    pos_of_node = np.full(n, -1, np.int64)
    for c in range(NCORES):
        for bi, bn in enumerate(core_bins[c]):
            for j, nd in enumerate(bn):
                if nd >= 0:
                    slot2node[c, bi * BINCAP + j] = nd
                    pos_of_node[nd] = c * NSLOT + bi * BINCAP + j
    assert (pos_of_node >= 0).all()
    s.slot2node, s.pos_of_node = slot2node, pos_of_node
    s.SPLIT = 4 * NSLOT

    dst_pos = pos_of_node[dst]
    dst_core = dst_pos // NSLOT
    dst_bin = (dst_pos % NSLOT) // BINCAP
    dst_slot = (dst_pos % NSLOT) % BINCAP
    src_pos = pos_of_node[src]

    NT_A, NT_B = NB * TA, NB * TB
    idxA = np.zeros((NCORES, NT_A * 128), np.int16)
    dstA = np.full((NCORES, NT_A * 128), -1.0, np.float32)
    idxB = np.zeros((NCORES, NT_B * 128), np.int16)
    dstB = np.full((NCORES, NT_B * 128), -1.0, np.float32)

    order = np.lexsort((src_pos, dst_bin, dst_core))
    eo_src, eo_core = src_pos[order], dst_core[order]
    eo_bin, eo_slot, eo_a = dst_bin[order], dst_slot[order], a_edge[order]

    for c in range(NCORES):
        msk_c = eo_core == c
        for idxarr, dstarr, T, off, grp in (
            (idxA, dstA, TA, 0, True),
            (idxB, dstB, TB, s.SPLIT, False),
        ):
            msk = msk_c & (eo_a == grp)
            bins_e, srcs, slots = eo_bin[msk], eo_src[msk] - off, eo_slot[msk]
            bs = np.searchsorted(bins_e, np.arange(NB))
            be = np.searchsorted(bins_e, np.arange(NB), side="right")
            for bi in range(NB):
                k = be[bi] - bs[bi]
                assert k <= T * 128
                base = bi * T * 128
                idxarr[c, base : base + k] = srcs[bs[bi] : be[bi]].astype(np.int16)
                dstarr[c, base : base + k] = slots[bs[bi] : be[bi]].astype(np.float32)

    s.idxA, s.dstA, s.idxB, s.dstB = idxA, dstA, idxB, dstB

    valid = slot2node >= 0
    s.valid = valid
    s.dinv_slot = np.where(
        valid, s.dinv_node[np.clip(slot2node, 0, None)], 0.0
    ).astype(np.float32)
    s.mask_slot = valid.astype(np.float32)

    maxcnt = int(cnt.max())
    SG = max(64, -(-maxcnt // 64) * 64)   # %64 so 2-graph pool gathers are %128
    s.SG = SG
    gidx_mean = np.zeros((NCORES, GPC * SG), np.int16)
    gidx_max = np.zeros((NCORES, GPC * SG), np.int16)
    for c in range(NCORES):
        for gl in range(GPC):
            g = core_graphs[c][gl]
            slots = (pos_of_node[np.arange(gstart[g], gend[g])] % NSLOT).astype(
                np.int16
            )
            base = gl * SG
            gidx_mean[c, base : base + len(slots)] = slots
            gidx_max[c, base : base + len(slots)] = slots
            gidx_mean[c, base + len(slots) : base + SG] = 1
            gidx_max[c, base + len(slots) : base + SG] = 0
    s.gidx_mean, s.gidx_max = gidx_mean, gidx_max
    cnt_perm = cnt[s.graph_order]
    s.inv_cnt = (1.0 / np.maximum(cnt_perm, 1)).astype(np.float32)
    s.maxmask = (cnt_perm > 0).astype(np.float32)
    return s


def fold_weights(w):
    f = {}
    w32 = {k: np.asarray(v, np.float32) if np.asarray(v).dtype != np.int64 else v
           for k, v in w.items()}
    sbn1 = w32["bn1_g"] / np.sqrt(1.0 + EPS_BN)
    f["Wc"] = (w32["conv1_W"] * sbn1[None, :]).astype(np.float32)
    f["btot_conv"] = (w32["conv1_b"] * sbn1 + w32["bn1_b"]).astype(np.float32)
    f["ln_g"], f["ln_b"] = w32["ln_g"], w32["ln_b"]
    f["prelu_a"], f["gen_t"] = w32["prelu_a"], w32["gen_t"]
    f["W1"], f["b1tot"], f["W2"], f["b2"] = [], [], [], []
    for i in range(L):
        smlp = w32["mlp_bn_g"][i] / np.sqrt(1.0 + EPS_BN)
        f["W1"].append((w32["mlp_W1"][i] * smlp[None, :]).astype(np.float32))
        f["b1tot"].append(
            (w32["mlp_b1"][i] * smlp + w32["mlp_bn_b"][i]).astype(np.float32)
        )
        f["W2"].append(w32["mlp_W2"][i])
        f["b2"].append(w32["mlp_b2"][i])
    for k in ("lin1_W", "lin1_b", "lin2_W", "lin2_b", "out_W", "out_b"):
        f[k] = w32[k]
    return f


def _wrap16(arr):
    """[K*16] -> [128, K] gather-idx layout (i at [i%16, i//16], tiled x8)."""
    a = np.asarray(arr, np.int16).reshape(-1, 16).T  # [16, K]
    return np.tile(a, (8, 1)).copy()


def _tile_major(arr, ntiles):
    """[ntiles*128] -> [128, ntiles] (partition = slot within tile)."""
    return np.ascontiguousarray(np.asarray(arr).reshape(ntiles, 128).T)


def build_inmaps(s, x):
    n = x.shape[0]
    NSLOT, NBLK = s.NSLOT, s.NBLK
    maps = []
    for c in range(NCORES):
        xpad = np.zeros((NSLOT, F_IN), np.float32)
        v = s.valid[c]
        xpad[v] = np.asarray(x, np.float32)[s.slot2node[c][v]]
        m = {
            "xT": np.ascontiguousarray(xpad.T),
            "idxA": _wrap16(s.idxA[c]),
            "idxB": _wrap16(s.idxB[c]),
            "dstA16": _tile_major(s.dstA[c], s.NB * TA).astype(NPBF),
            "dstB16": _tile_major(s.dstB[c], s.NB * TB).astype(NPBF),
            "dinv": np.ascontiguousarray(
                s.dinv_slot[c].reshape(NBLK, 128).T
            ),
            "mask": np.ascontiguousarray(
                s.mask_slot[c].reshape(NBLK, 128).T
            ),
            "gidxm": _wrap16(s.gidx_mean[c]),
            "gidxx": _wrap16(s.gidx_max[c]),
            "pminv": np.tile(s.inv_cnt[c * s.GPC : (c + 1) * s.GPC], (128, 1)).astype(np.float32),
            "pmax": np.tile(s.maxmask[c * s.GPC : (c + 1) * s.GPC], (128, 1)).astype(np.float32),
        }
        maps.append(m)
    return maps


# ---------------------------------------------------------------- bass build
def build_nc(s, f):
    NB, NSLOT, NBLK, SG, GPC = s.NB, s.NSLOT, s.NBLK, s.SG, s.GPC
    NSC = NB // CHUNK_BINS
    NT_CH_A = CHUNK_BINS * TA            # tiles per A-chunk (48)
    NT_CH_B = CHUNK_BINS * TB
    NIDX_A = NT_CH_A * 128
    NIDX_B = NT_CH_B * 128
    NTA, NTB = NB * TA, NB * TB

    nc = bacc.Bacc(get_trn_type() or "TRN2", num_devices=NCORES, num_swdge_queues=2,
                   dynamic_dma_scratch_size=DMA_SCRATCH)

    # ---- I/O ----
    xT_d = nc.dram_tensor("xT", [F_IN, NSLOT], F32, kind="ExternalInput")
    idxA_d = nc.dram_tensor("idxA", [128, NTA * 8], I16, kind="ExternalInput")
    idxB_d = nc.dram_tensor("idxB", [128, NTB * 8], I16, kind="ExternalInput")
    dstA16_d = nc.dram_tensor("dstA16", [128, NTA], BF16, kind="ExternalInput")
    dstB16_d = nc.dram_tensor("dstB16", [128, NTB], BF16, kind="ExternalInput")
    dinv_d = nc.dram_tensor("dinv", [128, NBLK], F32, kind="ExternalInput")
    mask_d = nc.dram_tensor("mask", [128, NBLK], F32, kind="ExternalInput")
    gidxm_d = nc.dram_tensor("gidxm", [128, GPC * SG // 16], I16, kind="ExternalInput")
    gidxx_d = nc.dram_tensor("gidxx", [128, GPC * SG // 16], I16, kind="ExternalInput")
    pminv_d = nc.dram_tensor("pminv", [128, GPC], F32, kind="ExternalInput")
    pmax_d = nc.dram_tensor("pmax", [128, GPC], F32, kind="ExternalInput")
    out_d = nc.dram_tensor("out", [s.G, 1], F32, kind="ExternalOutput")

    # ---- shared consts ----
    it = nc.inline_tensor
    Wc_d = it(f["Wc"], "Wc")                                     # [5,64]
    btotb_d = it(np.tile(f["btot_conv"], (128, 1)), "btotb")     # [128,64]
    W1_d = [it(f["W1"][i], f"W1_{i}") for i in range(L)]         # [64,128]
    W2_d = [it(f["W2"][i], f"W2_{i}") for i in range(L)]         # [128,64]
    b1_d = [it(f["b1tot"][i][:, None], f"b1_{i}") for i in range(L)]   # [128,1]
    b2b_d = [it(np.tile(f["b2"][i], (128, 1)), f"b2b_{i}") for i in range(L)]
    gbb_d = [it(np.tile(f["ln_g"][i], (128, 1)), f"gbb_{i}") for i in range(L)]
    bbb_d = [it(np.tile(f["ln_b"][i], (128, 1)), f"bbb_{i}") for i in range(L)]
    abb_d = [it(np.tile(f["prelu_a"][i], (128, 1)), f"abb_{i}") for i in range(L)]
    l1W_d = [it(np.ascontiguousarray(f["lin1_W"][k * 128 : (k + 1) * 128]), f"l1W_{k}") for k in range(4)]
    l1b_d = it(f["lin1_b"][:, None], "l1b")                      # [128,1]
    l2W_d = it(f["lin2_W"], "l2W")                               # [128,64]
    l2b_d = it(f["lin2_b"][:, None], "l2b")                      # [64,1]
    oW_d = it(f["out_W"], "oW")                                  # [64,1]
    iotab_d = it(np.tile(np.arange(32, dtype=np.float32), (128, 1)).astype(NPBF), "iotab")
    ident_d = it(np.eye(128, dtype=np.float32), "ident")

    # ---- internal DRAM ----
    ag_in = nc.dram_tensor("ag_in", [NSLOT, 2 * H], BF16)
    ag_out = nc.dram_tensor("ag_out", [NCORES * NSLOT, 2 * H], BF16, addr_space="Shared")
    pool_in = nc.dram_tensor("pool_in", [4, 128, GPC], F32)
    pool_out = nc.dram_tensor("pool_out", [NCORES, 4, 128, GPC], F32, addr_space="Shared")

    RG = [list(range(NCORES))]

    def allgather(cin, cout):
        if MOCK_COLLECTIVES:
            nc.sync.dma_start(out=cout[0 : cin.shape[0]], in_=cin[:])
        else:
            nc.gpsimd.collective_compute(
                "AllGather", ALU.bypass, replica_groups=RG,
                ins=[cin[:]], outs=[cout[:]],
            )

    with tile.TileContext(nc) as tc:
        with tc.tile_pool(name="persist", bufs=1) as pp:
            # resident per-core data
            idxA_sb = pp.tile([128, NTA * 8], I16)
            nc.sync.dma_start(out=idxA_sb[:], in_=idxA_d[:, :])
            idxB_sb = pp.tile([128, NTB * 8], I16)
            nc.sync.dma_start(out=idxB_sb[:], in_=idxB_d[:, :])
            dstA16 = pp.tile([128, NTA], BF16)
            nc.sync.dma_start(out=dstA16[:], in_=dstA16_d[:, :])
            dstB16 = pp.tile([128, NTB], BF16)
            nc.sync.dma_start(out=dstB16[:], in_=dstB16_d[:, :])
            dinv = pp.tile([128, NBLK], F32)
            nc.sync.dma_start(out=dinv[:], in_=dinv_d[:, :])
            mask = pp.tile([128, NBLK], F32)
            nc.sync.dma_start(out=mask[:], in_=mask_d[:, :])

            # consts
            _ldn = [0]

            def ld(dram, shape, dtype=F32):
                _ldn[0] += 1
                nm = f"c{_ldn[0]}_{dram.name}"
                t = pp.tile(shape, dtype, name=nm, tag=nm)
                nc.sync.dma_start(out=t[:], in_=dram[tuple(slice(None) for _ in shape)])
                return t

            Wc = ld(Wc_d, [F_IN, H])
            btotb = ld(btotb_d, [128, H])
            W1 = [ld(W1_d[i], [H, 2 * H]) for i in range(L)]
            W2 = [ld(W2_d[i], [2 * H, H]) for i in range(L)]
            b1 = [ld(b1_d[i], [128, 1]) for i in range(L)]
            b2b = [ld(b2b_d[i], [128, H]) for i in range(L)]
            gbb = [ld(gbb_d[i], [128, H]) for i in range(L)]
            bbb = [ld(bbb_d[i], [128, H]) for i in range(L)]
            abb = [ld(abb_d[i], [128, H]) for i in range(L)]
            l1W = [ld(l1W_d[k], [128, 128]) for k in range(4)]
            l1b = ld(l1b_d, [128, 1])
            l2W = ld(l2W_d, [128, H])
            l2b = ld(l2b_d, [H, 1])
            oW = ld(oW_d, [H, 1])
            iotab = ld(iotab_d, [128, 32], BF16)
            ident = ld(ident_d, [128, 128])
            epsb = pp.tile([128, 1], F32)
            nc.vector.memset(epsb[:], EPS_BN)

            # persistent state
            ledger = pp.tile([128, NBLK, (L + 1) * H], F32)
            usc = pp.tile([128, NBLK, H], F32)       # h0n during conv, u in GEN
            ab = pp.tile([128, NBLK, 2 * H], BF16)

            assert NIDX_A == NIDX_B
            nidx_subreg = nc.gpsimd.to_reg(NIDX_A // GATHER_SPLIT)

            def edge_phase(tag, drain_fn, mid_hook=None):
                """Shared edge machinery over ag_out. drain_fn(blk, psum_tile);
                mid_hook() fires after the half-way block's drain."""
                table_dram, nch = ag_out, 2 * H
                table_dtype = BF16
                dstA_t, dstB_t = dstA16, dstB16
                sdt = BF16
                with (
                    tc.tile_pool(name=f"ep_{tag}", bufs=1) as ep,
                    tc.tile_pool(name=f"epp_{tag}", bufs=2, space="PSUM") as epp,
                    tc.tile_pool(name=f"mpp_{tag}", bufs=2, space="PSUM") as mpp,
                ):
                    for sc in range(NSC):
                        ia = idxA_sb[:, sc * (NIDX_A // 16) : (sc + 1) * (NIDX_A // 16)]
                        ib = idxB_sb[:, sc * (NIDX_B // 16) : (sc + 1) * (NIDX_B // 16)]
                        ga = ep.tile([128, NT_CH_A, nch], table_dtype, tag="ga", bufs=2)
                        gb = ep.tile([128, NT_CH_B, nch], table_dtype, tag="gb", bufs=2)
                        if EDGE_GATHER:
                            GS = GATHER_SPLIT
                            tpc = NT_CH_A // GS      # tiles per sub-call
                            nn = tpc * 128
                            for k in range(GS):
                                nc.gpsimd.dma_gather(
                                    ga[:, k * tpc : (k + 1) * tpc, :],
                                    table_dram[0 : s.SPLIT, :],
                                    ia[:, k * (nn // 16) : (k + 1) * (nn // 16)],
                                    nn, nidx_subreg, nch,
                                    queue_num=0,
                                )
                                nc.gpsimd.dma_gather(
                                    gb[:, k * tpc : (k + 1) * tpc, :],
                                    table_dram[s.SPLIT : 2 * s.SPLIT, :],
                                    ib[:, k * (nn // 16) : (k + 1) * (nn // 16)],
                                    nn, nidx_subreg, nch,
                                    queue_num=1,
                                )
                        else:
                            nc.vector.memset(ga[:], 0.25)
                            nc.vector.memset(gb[:], 0.25)
                        sa = ep.tile([128, NT_CH_A, 32], sdt, tag="sa", bufs=2)
                        iot = iotab
                        nc.vector.tensor_tensor(
                            out=sa[:],
                            in0=dstA_t[:, sc * NT_CH_A : (sc + 1) * NT_CH_A]
                            .unsqueeze(2).broadcast_to([128, NT_CH_A, 32]),
                            in1=iot[:].unsqueeze(1).broadcast_to([128, NT_CH_A, 32]),
                            op=ALU.is_equal,
                        )
                        sb = ep.tile([128, NT_CH_B, 32], sdt, tag="sb", bufs=2)
                        nc.vector.tensor_tensor(
                            out=sb[:],
                            in0=dstB_t[:, sc * NT_CH_B : (sc + 1) * NT_CH_B]
                            .unsqueeze(2).broadcast_to([128, NT_CH_B, 32]),
                            in1=iot[:].unsqueeze(1).broadcast_to([128, NT_CH_B, 32]),
                            op=ALU.is_equal,
                        )
                        for bl in range(CHUNK_BINS // 4):
                            blk = sc * (CHUNK_BINS // 4) + bl
                            ps = epp.tile([128, nch], F32, tag="eps", space="PSUM")
                            if not EDGE_MM:
                                nc.vector.memset(ps[:], 0.0)
                                drain_fn(blk, ps, ep, mpp)
                                continue
                            for j in range(4):
                                lbin = bl * 4 + j       # bin within superchunk
                                for t in range(TA):
                                    nc.tensor.matmul(
                                        out=ps[32 * j : 32 * j + 32, :],
                                        lhsT=sa[:, lbin * TA + t, :],
                                        rhs=ga[:, lbin * TA + t, :],
                                        start=(t == 0),
                                        stop=False,
                                        tile_position=(0, 32 * j),
                                    )
                                for t in range(TB):
                                    nc.tensor.matmul(
                                        out=ps[32 * j : 32 * j + 32, :],
                                        lhsT=sb[:, lbin * TB + t, :],
                                        rhs=gb[:, lbin * TB + t, :],
                                        start=False,
                                        stop=(t == TB - 1),
                                        tile_position=(0, 32 * j),
                                    )
                            drain_fn(blk, ps, ep, mpp)
                            if mid_hook is not None and blk == HALFB - 1:
                                mid_hook()

            CH = (L + 1) * H
            lbf = pp.tile([128, NBLK, CH], BF16)     # bf16 pooling copy

            # ---- lagged batched node phase ----
            # Drains only collect LN stats per block (DVE); every G8 blocks a
            # batched tail computes LN -> PReLU -> u (usc) and the softmax
            # table rows [exp(t*v), v*exp(t*v)] into ab (bf16) as 3D group
            # ops. Batching keeps Sqrt/Exp activation-table reloads to 2 per
            # group instead of 2 per block, and it all overlaps the gathers.
            G8 = 8
            mvall = pp.tile([128, NBLK, 2], F32)

            def node_stats(i, blk, ep):
                h = ledger[:, blk, i * H : (i + 1) * H]
                st = ep.tile([128, 6], F32, tag="nst", bufs=3)
                nc.vector.bn_stats(out=st[:], in_=h)
                nc.vector.bn_aggr(out=mvall[:, blk, :], in_=st[:])

            def node_tail(i, lo, hi, ep):
                W = hi - lo
                rstd = ep.tile([128, G8], F32, tag="nrstd", bufs=2)
                nc.scalar.activation(
                    out=rstd[:, 0:W], in_=mvall[:, lo:hi, 1], func=AF.Sqrt,
                    bias=epsb[:], scale=1.0,
                )
                nc.vector.reciprocal(out=rstd[:, 0:W], in_=rstd[:, 0:W])
                nmr = ep.tile([128, G8], F32, tag="nnmr", bufs=2)
                nc.vector.tensor_tensor(
                    out=nmr[:, 0:W], in0=mvall[:, lo:hi, 0], in1=rstd[:, 0:W],
                    op=ALU.mult,
                )
                nc.vector.tensor_scalar(
                    out=nmr[:, 0:W], in0=nmr[:, 0:W], scalar1=-1.0,
                    scalar2=None, op0=ALU.mult,
                )
                hsl = ledger[:, lo:hi, i * H : (i + 1) * H]
                u = usc[:, lo:hi, :]
                rb = rstd[:, 0:W].unsqueeze(2).broadcast_to([128, W, H])
                nb_ = nmr[:, 0:W].unsqueeze(2).broadcast_to([128, W, H])
                gbig = gbb[i][:].unsqueeze(1).broadcast_to([128, W, H])
                bbig = bbb[i][:].unsqueeze(1).broadcast_to([128, W, H])
                abig = abb[i][:].unsqueeze(1).broadcast_to([128, W, H])
                nc.vector.tensor_tensor(out=u, in0=hsl, in1=rb, op=ALU.mult)
                nc.vector.tensor_tensor(out=u, in0=u, in1=nb_, op=ALU.add)
                nc.vector.tensor_tensor(out=u, in0=u, in1=gbig, op=ALU.mult)
                nc.vector.tensor_tensor(out=u, in0=u, in1=bbig, op=ALU.add)
                r = ep.tile([128, G8, H], F32, tag="nrl", bufs=2)
                nc.vector.tensor_scalar(
                    out=r[:, 0:W, :], in0=u, scalar1=0.0, scalar2=None,
                    op0=ALU.max,
                )
                mneg = ep.tile([128, G8, H], F32, tag="nmg", bufs=2)
                nc.vector.tensor_tensor(
                    out=mneg[:, 0:W, :], in0=u, in1=r[:, 0:W, :], op=ALU.subtract
                )
                nc.vector.tensor_tensor(
                    out=mneg[:, 0:W, :], in0=mneg[:, 0:W, :], in1=abig, op=ALU.mult
                )
                nc.vector.tensor_add(u, r[:, 0:W, :], mneg[:, 0:W, :])
                vb = ep.tile([128, G8, H], F32, tag="nvb", bufs=2)
                nc.vector.tensor_scalar(
                    out=vb[:, 0:W, :], in0=u, scalar1=0.0, scalar2=EPS_MSG,
                    op0=ALU.max, op1=ALU.add,
                )
                Ab = ep.tile([128, G8, H], F32, tag="nAb", bufs=2)
                nc.scalar.activation(
                    out=Ab[:, 0:W, :], in_=vb[:, 0:W, :], func=AF.Exp,
                    scale=float(f["gen_t"][i]),
                )
                nc.vector.tensor_copy(out=ab[:, lo:hi, 0:H], in_=Ab[:, 0:W, :])
                nc.vector.tensor_tensor(
                    out=ab[:, lo:hi, H : 2 * H], in0=vb[:, 0:W, :],
                    in1=Ab[:, 0:W, :], op=ALU.mult,
                )

            def node_step(i, blk, ep):
                node_stats(i, blk, ep)
                if (blk + 1) % G8 == 0 or blk == NBLK - 1:
                    node_tail(i, blk - (blk % G8), blk + 1, ep)

            def pool_prep(blk, ep):
                if blk == 0:
                    nc.vector.memset(ledger[0:1, 0, 0:CH], -3.0e38)
                nc.vector.tensor_copy(out=lbf[:, blk, :], in_=ledger[:, blk, :])

            # ================= conv =================
            with (
                tc.tile_pool(name="cvp", bufs=4, space="PSUM") as cvp,
                tc.tile_pool(name="cvs", bufs=1) as cvs,
            ):
                # one DMA for all of x; the cvs pool closes before the edge
                # pools open, so the tile doesn't stack with gather buffers
                xt_all = cvs.tile([F_IN, NSLOT], F32, tag="xt_all")
                nc.sync.dma_start(out=xt_all[:], in_=xT_d[:, :])
                nc.vector.memset(ab[:, :, H : 2 * H], 0.0)
                for blk in range(NBLK):
                    h0ps = cvp.tile([128, H], F32, space="PSUM")
                    nc.tensor.matmul(
                        out=h0ps[:],
                        lhsT=xt_all[:, blk * 128 : (blk + 1) * 128],
                        rhs=Wc[:],
                        start=True, stop=True,
                    )
                    nc.vector.tensor_scalar(
                        out=usc[:, blk, :], in0=h0ps[:],
                        scalar1=dinv[:, blk : blk + 1], scalar2=None,
                        op0=ALU.mult,
                    )
                nc.vector.tensor_copy(out=ab[:, :, 0:H], in_=usc[:])

            def conv_drain(blk, ps, ep, mpp):
                t1 = ep.tile([128, H], F32, tag="cd", bufs=3)
                nc.vector.tensor_add(t1[:], ps[:, 0:H], usc[:, blk, :])
                nc.vector.tensor_scalar(
                    out=t1[:], in0=t1[:],
                    scalar1=dinv[:, blk : blk + 1], scalar2=None, op0=ALU.mult,
                )
                nc.vector.tensor_add(t1[:], t1[:], btotb[:])
                nc.vector.tensor_scalar(
                    out=ledger[:, blk, 0:H], in0=t1[:],
                    scalar1=0.0, scalar2=mask[:, blk : blk + 1],
                    op0=ALU.max, op1=ALU.mult,
                )
                node_step(0, blk, ep)

            # ================= edge phases =================
            # conv table (= [h0n | 0] bf16) is in ab; each GEN layer's drain
            # writes ledger[i+1] and fuses the next node phase (or pool prep).
            def make_gen_drain(i):
                def gen_drain(blk, ps, ep, mpp):
                    sden = ep.tile([128, H], F32, tag="sden", bufs=3)
                    nc.vector.tensor_scalar(
                        out=sden[:], in0=ps[:, 0:H], scalar1=1e-30, scalar2=None,
                        op0=ALU.add,
                    )
                    nc.vector.reciprocal(out=sden[:], in_=sden[:])
                    agg = ep.tile([128, H], F32, tag="agg", bufs=3)
                    nc.vector.tensor_tensor(
                        out=agg[:], in0=ps[:, H : 2 * H], in1=sden[:], op=ALU.mult
                    )
                    nc.vector.tensor_add(agg[:], agg[:], usc[:, blk, :])
                    tps = mpp.tile([H, 128], F32, tag="tps", space="PSUM")
                    nc.tensor.transpose(out=tps[:], in_=agg[:], identity=ident[:])
                    aggT = ep.tile([H, 128], F32, tag="aggT", bufs=3)
                    nc.vector.tensor_copy(out=aggT[:], in_=tps[:])
                    z1ps = mpp.tile([128, 128], F32, tag="z1", space="PSUM")
                    nc.tensor.matmul(
                        out=z1ps[:], lhsT=W1[i][:], rhs=aggT[:], start=True, stop=True
                    )
                    z1r = ep.tile([128, 128], F32, tag="z1r", bufs=3)
                    nc.scalar.activation(
                        out=z1r[:], in_=z1ps[:], func=AF.Relu, bias=b1[i][:], scale=1.0
                    )
                    z2ps = mpp.tile([128, H], F32, tag="z2", space="PSUM")
                    nc.tensor.matmul(
                        out=z2ps[:], lhsT=z1r[:], rhs=W2[i][:], start=True, stop=True
                    )
                    t2 = ep.tile([128, H], F32, tag="t2", bufs=3)
                    nc.vector.tensor_add(t2[:], z2ps[:], b2b[i][:])
                    nc.vector.tensor_add(t2[:], t2[:], ledger[:, blk, i * H : (i + 1) * H])
                    nc.vector.tensor_scalar(
                        out=ledger[:, blk, (i + 1) * H : (i + 2) * H], in0=t2[:],
                        scalar1=mask[:, blk : blk + 1], scalar2=None, op0=ALU.mult,
                    )
                    if i + 1 < L:
                        node_step(i + 1, blk, ep)
                    else:
                        pool_prep(blk, ep)
                return gen_drain

            HALFB = -(-(NBLK // 2) // G8) * G8   # G8-aligned: ab half complete

            def ab_dma(lo, hi):
                nc.sync.dma_start(
                    out=ag_in.ap().rearrange("(b p) c -> p b c", p=128)[:, lo:hi, :],
                    in_=ab[:, lo:hi, :],
                )

            drains = [conv_drain] + [make_gen_drain(i) for i in range(L)]
            for phase in range(L + 1):
                if phase == 0:
                    ab_dma(0, NBLK)       # conv table written during conv compute
                else:
                    ab_dma(HALFB, NBLK)   # first half went out mid prior phase
                allgather(ag_in, ag_out)
                mid = (lambda: ab_dma(0, HALFB)) if phase < L else None
                edge_phase(f"e{phase}", drains[phase], mid_hook=mid)

            # ================= pooling + head =================
            if PHASES < 3:
                dbg = nc.dram_tensor("dbg", [128, NBLK, (L + 1) * H], F32,
                                     kind="ExternalOutput")
                nc.sync.dma_start(out=dbg[:, :, :], in_=ledger[:])
            from contextlib import ExitStack as _ES
            with _ES() as _pool_ctx:
              if PHASES >= 3:
                qp = _pool_ctx.enter_context(tc.tile_pool(name="pool", bufs=1))
                qpp = _pool_ctx.enter_context(
                    tc.tile_pool(name="poolps", bufs=2, space="PSUM")
                )
                GPER = 2                          # graphs per pool-gather call
                PGS = GPER * SG                   # idxs per sub-call
                gnidx_reg = nc.gpsimd.to_reg(PGS)
                pooled = qp.tile([128, 4, GPC], F32)
                for which, gidx_d, pscale_d in (
                    (0, gidxm_d, pminv_d),
                    (1, gidxx_d, pmax_d),
                ):
                    gi = qp.tile([128, GPC * SG // 16], I16, tag="gi", bufs=2)
                    nc.sync.dma_start(out=gi[:], in_=gidx_d[:, :])
                    psc = qp.tile([128, GPC], F32, tag="psc", bufs=2)
                    nc.sync.dma_start(out=psc[:], in_=pscale_d[:, :])
                    redw = qp.tile([128, 2, GPC], F32, tag="redw", bufs=2)
                    red_op = nc.vector.reduce_sum if which == 0 else nc.vector.reduce_max
                    for k in range(GPC // GPER):
                        grid = qp.tile([128, 2, PGS], BF16, tag="grid", bufs=2)
                        nc.gpsimd.dma_gather(
                            grid[:, :, :],
                            lbf[:].rearrange("p b c -> p (b c)"),
                            gi[:, k * (PGS // 16) : (k + 1) * (PGS // 16)],
                            PGS, gnidx_reg, CH,
                            transpose=True,
                            sbuf_tokens_per_rank=128,
                            sbuf_free_dim_per_rank=CH * 2,
                            queue_num=k % 2,
                        )
                        for half in range(2):
                            red_op(
                                out=redw[:, half, k * GPER : (k + 1) * GPER],
                                in_=grid[:, half, :].rearrange(
                                    "p (m t) -> p m t", t=SG
                                ),
                                axis=mybir.AxisListType.X,
                            )
                    for half in range(2):
                        nc.vector.tensor_tensor(
                            out=pooled[:, which * 2 + half, :], in0=redw[:, half, :],
                            in1=psc[:], op=ALU.mult,
                        )
                nc.sync.dma_start(
                    out=pool_in.ap().rearrange("k p g -> p k g"), in_=pooled[:]
                )
                if MOCK_COLLECTIVES:
                    nc.sync.dma_start(
                        out=pool_out[0, :, :, :], in_=pool_in[:, :, :]
                    )
                else:
                    nc.gpsimd.collective_compute(
                        "AllGather", ALU.bypass, replica_groups=RG,
                        ins=[pool_in[:, :, :]], outs=[pool_out[:, :, :, :]],
                    )
                # head
                hps = qpp.tile([128, s.G], F32, tag="hps", space="PSUM")
                pk = []
                for k in range(4):
                    t = qp.tile([128, NCORES, GPC], F32, tag=f"pk{k}")
                    nc.sync.dma_start(
                        out=t[:], in_=pool_out[:, k, :, :].rearrange("r p g -> p r g")
                    )
                    pk.append(t)
                for k in range(4):
                    nc.tensor.matmul(
                        out=hps[:], lhsT=l1W[k][:],
                        rhs=pk[k][:].rearrange("p r g -> p (r g)"),
                        start=(k == 0), stop=(k == 3),
                    )
                hz1 = qp.tile([128, s.G], F32)
                nc.scalar.activation(
                    out=hz1[:], in_=hps[:], func=AF.Relu, bias=l1b[:], scale=1.0
                )
                h2ps = qpp.tile([H, s.G], F32, tag="h2ps", space="PSUM")
                nc.tensor.matmul(out=h2ps[:], lhsT=l2W[:], rhs=hz1[:], start=True, stop=True)
                hz2 = qp.tile([H, s.G], F32)
                nc.scalar.activation(
                    out=hz2[:], in_=h2ps[:], func=AF.Relu, bias=l2b[:], scale=1.0
                )
                ops = qpp.tile([1, s.G], F32, tag="ops", space="PSUM")
                nc.tensor.matmul(out=ops[:], lhsT=oW[:], rhs=hz2[:], start=True, stop=True)
                osb = qp.tile([1, s.G], F32)
                nc.vector.tensor_scalar(
                    out=osb[:], in0=ops[:], scalar1=float(f["out_b"][0]),
                    scalar2=None, op0=ALU.add,
                )
                nc.sync.dma_start(out=out_d.ap().rearrange("g one -> one g"), in_=osb[:])

    nc.compile()
    return nc


def _insert_library_loads(nc):
    import bass_rust as _bass_rust
    from concourse.library_config import all_libraries, standard

    inst_type_to_lib_mask = {}
    for lib in all_libraries:
        for inst_type in lib.instructions:
            inst_type_to_lib_mask[inst_type] = inst_type_to_lib_mask.get(
                inst_type, 0
            ) | (1 << lib.index)
    _bass_rust.insert_library_loads(
        nc, inst_type_to_lib_mask, len(all_libraries), standard.index
    )


# ---------------------------------------------------------------- wait split
def split_waits(nc, max_waits: int = 1) -> int:
    nsplit = 0
    for fn in nc.m.functions:
        for bb in fn.blocks:
            new_insts = []
            for ins in bb.instructions:
                si = ins.sync_info
                if si is not None and si.on_wait and len(si.on_wait) > max_waits:
                    waits = list(si.on_wait)
                    spill, keep = waits[:-max_waits], waits[-max_waits:]
                    for k, w in enumerate(spill):
                        nop = mybir.InstNoOp(
                            name=f"{ins.name}-wsplit{k}",
                            engine=ins.engine,
                            bass_nofuse=True,
                            sync_info=mybir.SyncInfo(on_wait=[w], on_update=[]),
                        )
                        new_insts.append(nop)
                        nc.register_instruction(nop, overwrite=True)
                        nsplit += 1
                    si.on_wait = keep
                new_insts.append(ins)
            if len(new_insts) != len(bb.instructions):
                bb.instructions[:] = new_insts
    return nsplit


# ---------------------------------------------------------------- entry
def kernel(**inputs) -> np.ndarray:
    x = np.asarray(inputs["x"], np.float32)
    ei = np.asarray(inputs["edge_index"], np.int64)
    bi = np.asarray(inputs["batch_idx"], np.int64)
    G = 256
    s = build_schedule(ei, bi, G)
    f = fold_weights(inputs)
    maps = build_inmaps(s, x)
    nc = build_nc(s, f)
    res = run_bass_kernel_spmd(nc, maps, core_ids=list(range(NCORES)))
    raw = np.asarray(res.results[0]["out"], np.float32)
    out = np.empty_like(raw)
    out[s.graph_order] = raw              # undo the graph->core permutation
    return out



# revision 49
# speedup vs baseline: 1.2859x; 1.2859x over previous
"""Trainium2 Bass kernel for nn_GCN_5403068858882 (GCN + 3x GENConv + pool head).

Self-contained: schedule builder + bass program builder + SPMD runner.

- 8 cores; graphs are LPT-balanced across cores by edge count (host
  unpermutes the per-graph output rows afterwards).
- A core's nodes are LPT/worst-fit packed into 32-slot bins capped at 3*128
  "A" edges / 3*128 "B" edges (A = src core < 4 so int16 gather indices
  reach their table half).
- All four aggregations (GCN conv + 3x GENConv) share one bf16 edge
  machinery: node phase writes table rows [A|B] ([h0*dinv | 0] for conv,
  [exp(t*v), v*exp(t*v)] for GEN) -> AllGather -> per 128-edge tile:
  dma_gather rows (1024-idx calls; see cap note below) + PE matmul with an
  is_equal selection matrix accumulating per-dst sums in PSUM -> drain.
- Drains fuse everything downstream: GEN softmax agg=w/s+u, MLP (bn folded),
  residual ledger write, the NEXT layer's LN stats, and (lagged, per 8-block
  group, batched 3D ops) the next layer's LN/PReLU/exp table build — so the
  inter-layer "node phase" almost fully overlaps the gather stream.
- Pooling: bf16 SBUF-source dma_gather (transpose) into a per-graph padded
  channel-major grid -> one reduce per stat -> tiny AllGather -> MLP head.
"""

import numpy as np
import ml_dtypes

import concourse.bass as bass
import concourse.bacc as bacc


import concourse.mybir as mybir
import concourse.tile as tile
from concourse.bass_utils import run_bass_kernel_spmd
from concourse._compat import get_trn_type

F32 = mybir.dt.float32
BF16 = mybir.dt.bfloat16
I16 = mybir.dt.int16
AF = mybir.ActivationFunctionType
ALU = mybir.AluOpType
NPBF = ml_dtypes.bfloat16

H = 64
F_IN = 5
L = 3
EPS_BN = 1e-5
EPS_MSG = 1e-7
NCORES = 8
TA = 3
TB = 3
BINCAP = 32
CHUNK_BINS = 8           # bins per gather superchunk
MOCK_COLLECTIVES = False  # replace AllGathers with local DMA (TimelineSim)
PHASES = 3               # debug: 1=conv only, 2=+GEN layers, 3=+pool/head
CONV_AG = True           # debug: run the conv AllGather
CONV_EDGE = True         # debug: run the conv edge phase
EDGE_GATHER = True       # debug: issue dma_gather calls
EDGE_MM = True           # debug: issue edge matmuls
# Per-call SWDGE descriptor cap: ucode dge_n_inflight=128 per direction;
# a gather call generates ~num_idxs/16+1 descs per direction (transpose
# gathers ~num_idxs/8 s2m), so edge calls must stay <=2032 idxs and pool
# transpose calls <=~1000 idxs or the device hangs.
GATHER_SPLIT = 3         # 1024-idx edge calls -> 65 descs/dir
DMA_SCRATCH = 16384      # default SWDGE ring (plenty: ring slots = ndesc)


# ---------------------------------------------------------------- schedule
class Sched:
    pass


def build_schedule(edge_index, batch_idx, G):
    s = Sched()
    src = np.asarray(edge_index[0], np.int64)
    dst = np.asarray(edge_index[1], np.int64)
    batch = np.asarray(batch_idx, np.int64)
    n = batch.shape[0]
    s.G = G
    s.GPC = GPC = G // NCORES

    deg = np.bincount(dst, minlength=n).astype(np.float64) + 1.0
    s.dinv_node = (deg ** -0.5).astype(np.float32)

    gstart = np.searchsorted(batch, np.arange(G))
    gend = np.searchsorted(batch, np.arange(G), side="right")
    s.cnt = cnt = gend - gstart

    # Balance graphs across cores by dst-edge count (LPT): core c's 32 graphs
    # are core_graphs[c]; the host unpermutes output rows via graph_order.
    gedge = np.bincount(batch[dst], minlength=G)
    load = np.zeros(NCORES, np.int64)
    slots_left = np.full(NCORES, GPC, np.int64)
    core_graphs = [[] for _ in range(NCORES)]
    for g in np.argsort(-gedge, kind="stable"):
        ok = np.flatnonzero(slots_left > 0)
        c = int(ok[np.argmin(load[ok])])
        core_graphs[c].append(int(g))
        load[c] += gedge[g]
        slots_left[c] -= 1
    core_of_graph = np.empty(G, np.int64)
    for c in range(NCORES):
        for g in core_graphs[c]:
            core_of_graph[g] = c
    s.core_graphs = core_graphs
    s.graph_order = np.concatenate(
        [np.asarray(cg, np.int64) for cg in core_graphs]
    )

    a_edge = core_of_graph[batch[src]] < (NCORES // 2)
    acnt = np.bincount(dst[a_edge], minlength=n)
    bcnt = np.bincount(dst[~a_edge], minlength=n)

    # Balanced (LPT/worst-fit) packing: nodes within a core may occupy any bin
    # (pool gathers look slots up per-graph). Partition a core's nodes into nb
    # bins of <=BINCAP slots / <=CAP_A A-edges / <=CAP_B B-edges, descending
    # by load, each into the fitting bin with most remaining capacity.
    CAP_A, CAP_B = TA * 128, TB * 128

    def _pack_lpt(nds, nb, key):
        order_n = nds[np.argsort(-key, kind="stable")]
        rem_a = np.full(nb, CAP_A, np.int64)
        rem_b = np.full(nb, CAP_B, np.int64)
        rem_s = np.full(nb, BINCAP, np.int64)
        rem_s[0] -= 2                     # two reserved invalid slots in bin 0
        bins = [[] for _ in range(nb)]
        bins[0] = [-1, -1]
        for nd in order_n:
            a, b = acnt[nd], bcnt[nd]
            ok = np.flatnonzero((rem_a >= a) & (rem_b >= b) & (rem_s >= 1))
            if len(ok) == 0:
                return None
            bi = int(ok[np.argmax(rem_a[ok] + rem_b[ok])])
            bins[bi].append(int(nd))
            rem_a[bi] -= a
            rem_b[bi] -= b
            rem_s[bi] -= 1
        return bins

    core_bins = []
    for c in range(NCORES):
        nds = np.concatenate(
            [np.arange(gstart[g], gend[g]) for g in core_graphs[c]]
        )
        nb = max(
            -(-(len(nds) + 2) // BINCAP),
            -(-int(acnt[nds].sum()) // CAP_A),
            -(-int(bcnt[nds].sum()) // CAP_B),
        )
        keys = (
            acnt[nds] + bcnt[nds],
            np.maximum(acnt[nds], bcnt[nds]),
            2 * acnt[nds] + bcnt[nds],
            acnt[nds] + 2 * bcnt[nds],
        )
        bins = None
        while bins is None:
            for key in keys:
                bins = _pack_lpt(nds, nb, key)
                if bins is not None:
                    break
            else:
                nb += 1
        core_bins.append(bins)

    NB = max(len(b) for b in core_bins)
    NB = -(-NB // CHUNK_BINS) * CHUNK_BINS
    s.NB = NB
    s.NSLOT = NSLOT = NB * BINCAP
    s.NBLK = NB // 4
    assert 4 * NSLOT <= 32768, NSLOT

    slot2node = np.full((NCORES, NSLOT), -1, np.int64)
    pos_of_node = np.full(n, -1, np.int64)
    for c in range(NCORES):
        for bi, bn in enumerate(core_bins[c]):
            for j, nd in enumerate(bn):
                if nd >= 0:
                    slot2node[c, bi * BINCAP + j] = nd
                    pos_of_node[nd] = c * NSLOT + bi * BINCAP + j
    assert (pos_of_node >= 0).all()
    s.slot2node, s.pos_of_node = slot2node, pos_of_node
    s.SPLIT = 4 * NSLOT

    dst_pos = pos_of_node[dst]
    dst_core = dst_pos // NSLOT
    dst_bin = (dst_pos % NSLOT) // BINCAP
    dst_slot = (dst_pos % NSLOT) % BINCAP
    src_pos = pos_of_node[src]

    NT_A, NT_B = NB * TA, NB * TB
    idxA = np.zeros((NCORES, NT_A * 128), np.int16)
    dstA = np.full((NCORES, NT_A * 128), -1.0, np.float32)
    idxB = np.zeros((NCORES, NT_B * 128), np.int16)
    dstB = np.full((NCORES, NT_B * 128), -1.0, np.float32)

    order = np.lexsort((src_pos, dst_bin, dst_core))
    eo_src, eo_core = src_pos[order], dst_core[order]
    eo_bin, eo_slot, eo_a = dst_bin[order], dst_slot[order], a_edge[order]

    for c in range(NCORES):
        msk_c = eo_core == c
        for idxarr, dstarr, T, off, grp in (
            (idxA, dstA, TA, 0, True),
            (idxB, dstB, TB, s.SPLIT, False),
        ):
            msk = msk_c & (eo_a == grp)
            bins_e, srcs, slots = eo_bin[msk], eo_src[msk] - off, eo_slot[msk]
            bs = np.searchsorted(bins_e, np.arange(NB))
            be = np.searchsorted(bins_e, np.arange(NB), side="right")
            for bi in range(NB):
                k = be[bi] - bs[bi]
                assert k <= T * 128
                base = bi * T * 128
                idxarr[c, base : base + k] = srcs[bs[bi] : be[bi]].astype(np.int16)
                dstarr[c, base : base + k] = slots[bs[bi] : be[bi]].astype(np.float32)

    s.idxA, s.dstA, s.idxB, s.dstB = idxA, dstA, idxB, dstB

    valid = slot2node >= 0
    s.valid = valid
    s.dinv_slot = np.where(
        valid, s.dinv_node[np.clip(slot2node, 0, None)], 0.0
    ).astype(np.float32)
    s.mask_slot = valid.astype(np.float32)

    maxcnt = int(cnt.max())
    SG = max(64, -(-maxcnt // 64) * 64)   # %64 so 2-graph pool gathers are %128
    s.SG = SG
    gidx_mean = np.zeros((NCORES, GPC * SG), np.int16)
    gidx_max = np.zeros((NCORES, GPC * SG), np.int16)
    for c in range(NCORES):
        for gl in range(GPC):
            g = core_graphs[c][gl]
            slots = (pos_of_node[np.arange(gstart[g], gend[g])] % NSLOT).astype(
                np.int16
            )
            base = gl * SG
            gidx_mean[c, base : base + len(slots)] = slots
            gidx_max[c, base : base + len(slots)] = slots
            gidx_mean[c, base + len(slots) : base + SG] = 1
            gidx_max[c, base + len(slots) : base + SG] = 0
    s.gidx_mean, s.gidx_max = gidx_mean, gidx_max
    cnt_perm = cnt[s.graph_order]
    s.inv_cnt = (1.0 / np.maximum(cnt_perm, 1)).astype(np.float32)
    s.maxmask = (cnt_perm > 0).astype(np.float32)
    return s


def fold_weights(w):
    f = {}
    w32 = {k: np.asarray(v, np.float32) if np.asarray(v).dtype != np.int64 else v
           for k, v in w.items()}
    sbn1 = w32["bn1_g"] / np.sqrt(1.0 + EPS_BN)
    f["Wc"] = (w32["conv1_W"] * sbn1[None, :]).astype(np.float32)
    f["btot_conv"] = (w32["conv1_b"] * sbn1 + w32["bn1_b"]).astype(np.float32)
    f["ln_g"], f["ln_b"] = w32["ln_g"], w32["ln_b"]
    f["prelu_a"], f["gen_t"] = w32["prelu_a"], w32["gen_t"]
    f["W1"], f["b1tot"], f["W2"], f["b2"] = [], [], [], []
    for i in range(L):
        smlp = w32["mlp_bn_g"][i] / np.sqrt(1.0 + EPS_BN)
        f["W1"].append((w32["mlp_W1"][i] * smlp[None, :]).astype(np.float32))
        f["b1tot"].append(
            (w32["mlp_b1"][i] * smlp + w32["mlp_bn_b"][i]).astype(np.float32)
        )
        f["W2"].append(w32["mlp_W2"][i])
        f["b2"].append(w32["mlp_b2"][i])
    for k in ("lin1_W", "lin1_b", "lin2_W", "lin2_b", "out_W", "out_b"):
        f[k] = w32[k]
    return f


def _wrap16(arr):
    """[K*16] -> [128, K] gather-idx layout (i at [i%16, i//16], tiled x8)."""
    a = np.asarray(arr, np.int16).reshape(-1, 16).T  # [16, K]
    return np.tile(a, (8, 1)).copy()


def _tile_major(arr, ntiles):
    """[ntiles*128] -> [128, ntiles] (partition = slot within tile)."""
    return np.ascontiguousarray(np.asarray(arr).reshape(ntiles, 128).T)


def build_inmaps(s, x):
    n = x.shape[0]
    NSLOT, NBLK = s.NSLOT, s.NBLK
    maps = []
    for c in range(NCORES):
        xpad = np.zeros((NSLOT, F_IN), np.float32)
        v = s.valid[c]
        xpad[v] = np.asarray(x, np.float32)[s.slot2node[c][v]]
        m = {
            "xT": np.ascontiguousarray(xpad.T),
            "idxA": _wrap16(s.idxA[c]),
            "idxB": _wrap16(s.idxB[c]),
            "dstA16": _tile_major(s.dstA[c], s.NB * TA).astype(NPBF),
            "dstB16": _tile_major(s.dstB[c], s.NB * TB).astype(NPBF),
            "dinv": np.ascontiguousarray(
                s.dinv_slot[c].reshape(NBLK, 128).T
            ),
            "mask": np.ascontiguousarray(
                s.mask_slot[c].reshape(NBLK, 128).T
            ),
            "gidxm": _wrap16(s.gidx_mean[c]),
            "gidxx": _wrap16(s.gidx_max[c]),
            "pminv": np.tile(s.inv_cnt[c * s.GPC : (c + 1) * s.GPC], (128, 1)).astype(np.float32),
            "pmax": np.tile(s.maxmask[c * s.GPC : (c + 1) * s.GPC], (128, 1)).astype(np.float32),
        }
        maps.append(m)
    return maps


# ---------------------------------------------------------------- bass build
def build_nc(s, f):
    NB, NSLOT, NBLK, SG, GPC = s.NB, s.NSLOT, s.NBLK, s.SG, s.GPC
    NSC = NB // CHUNK_BINS
    NT_CH_A = CHUNK_BINS * TA            # tiles per A-chunk (48)
    NT_CH_B = CHUNK_BINS * TB
    NIDX_A = NT_CH_A * 128
    NIDX_B = NT_CH_B * 128
    NTA, NTB = NB * TA, NB * TB

    nc = bacc.Bacc(get_trn_type() or "TRN2", num_devices=NCORES, num_swdge_queues=2,
                   dynamic_dma_scratch_size=DMA_SCRATCH)

    # ---- I/O ----
    xT_d = nc.dram_tensor("xT", [F_IN, NSLOT], F32, kind="ExternalInput")
    idxA_d = nc.dram_tensor("idxA", [128, NTA * 8], I16, kind="ExternalInput")
    idxB_d = nc.dram_tensor("idxB", [128, NTB * 8], I16, kind="ExternalInput")
    dstA16_d = nc.dram_tensor("dstA16", [128, NTA], BF16, kind="ExternalInput")
    dstB16_d = nc.dram_tensor("dstB16", [128, NTB], BF16, kind="ExternalInput")
    dinv_d = nc.dram_tensor("dinv", [128, NBLK], F32, kind="ExternalInput")
    mask_d = nc.dram_tensor("mask", [128, NBLK], F32, kind="ExternalInput")
    gidxm_d = nc.dram_tensor("gidxm", [128, GPC * SG // 16], I16, kind="ExternalInput")
    gidxx_d = nc.dram_tensor("gidxx", [128, GPC * SG // 16], I16, kind="ExternalInput")
    pminv_d = nc.dram_tensor("pminv", [128, GPC], F32, kind="ExternalInput")
    pmax_d = nc.dram_tensor("pmax", [128, GPC], F32, kind="ExternalInput")
    out_d = nc.dram_tensor("out", [s.G, 1], F32, kind="ExternalOutput")

    # ---- shared consts ----
    it = nc.inline_tensor
    Wc_d = it(f["Wc"], "Wc")                                     # [5,64]
    btotb_d = it(np.tile(f["btot_conv"], (128, 1)), "btotb")     # [128,64]
    W1_d = [it(f["W1"][i], f"W1_{i}") for i in range(L)]         # [64,128]
    W2_d = [it(f["W2"][i], f"W2_{i}") for i in range(L)]         # [128,64]
    b1_d = [it(f["b1tot"][i][:, None], f"b1_{i}") for i in range(L)]   # [128,1]
    b2b_d = [it(np.tile(f["b2"][i], (128, 1)), f"b2b_{i}") for i in range(L)]
    gbb_d = [it(np.tile(f["ln_g"][i], (128, 1)), f"gbb_{i}") for i in range(L)]
    bbb_d = [it(np.tile(f["ln_b"][i], (128, 1)), f"bbb_{i}") for i in range(L)]
    abb_d = [it(np.tile(f["prelu_a"][i], (128, 1)), f"abb_{i}") for i in range(L)]
    l1W_d = [it(np.ascontiguousarray(f["lin1_W"][k * 128 : (k + 1) * 128]), f"l1W_{k}") for k in range(4)]
    l1b_d = it(f["lin1_b"][:, None], "l1b")                      # [128,1]
    l2W_d = it(f["lin2_W"], "l2W")                               # [128,64]
    l2b_d = it(f["lin2_b"][:, None], "l2b")                      # [64,1]
    oW_d = it(f["out_W"], "oW")                                  # [64,1]
    iotab_d = it(np.tile(np.arange(32, dtype=np.float32), (128, 1)).astype(NPBF), "iotab")
    ident_d = it(np.eye(128, dtype=np.float32), "ident")

    # ---- internal DRAM ----
    ag_in = nc.dram_tensor("ag_in", [NSLOT, 2 * H], BF16)
    ag_out = nc.dram_tensor("ag_out", [NCORES * NSLOT, 2 * H], BF16, addr_space="Shared")
    pool_in = nc.dram_tensor("pool_in", [4, 128, GPC], F32)
    pool_out = nc.dram_tensor("pool_out", [NCORES, 4, 128, GPC], F32, addr_space="Shared")

    RG = [list(range(NCORES))]

    def allgather(cin, cout):
        if MOCK_COLLECTIVES:
            nc.sync.dma_start(out=cout[0 : cin.shape[0]], in_=cin[:])
        else:
            nc.gpsimd.collective_compute(
                "AllGather", ALU.bypass, replica_groups=RG,
                ins=[cin[:]], outs=[cout[:]],
            )

    with tile.TileContext(nc) as tc:
        with tc.tile_pool(name="persist", bufs=1) as pp:
            # resident per-core data
            idxA_sb = pp.tile([128, NTA * 8], I16)
            nc.sync.dma_start(out=idxA_sb[:], in_=idxA_d[:, :])
            idxB_sb = pp.tile([128, NTB * 8], I16)
            nc.sync.dma_start(out=idxB_sb[:], in_=idxB_d[:, :])
            dstA16 = pp.tile([128, NTA], BF16)
            nc.sync.dma_start(out=dstA16[:], in_=dstA16_d[:, :])
            dstB16 = pp.tile([128, NTB], BF16)
            nc.sync.dma_start(out=dstB16[:], in_=dstB16_d[:, :])
            dinv = pp.tile([128, NBLK], F32)
            nc.sync.dma_start(out=dinv[:], in_=dinv_d[:, :])
            mask = pp.tile([128, NBLK], F32)
            nc.sync.dma_start(out=mask[:], in_=mask_d[:, :])

            # consts
            _ldn = [0]

            def ld(dram, shape, dtype=F32):
                _ldn[0] += 1
                nm = f"c{_ldn[0]}_{dram.name}"
                t = pp.tile(shape, dtype, name=nm, tag=nm)
                nc.sync.dma_start(out=t[:], in_=dram[tuple(slice(None) for _ in shape)])
                return t

            Wc = ld(Wc_d, [F_IN, H])
            btotb = ld(btotb_d, [128, H])
            W1 = [ld(W1_d[i], [H, 2 * H]) for i in range(L)]
            W2 = [ld(W2_d[i], [2 * H, H]) for i in range(L)]
            b1 = [ld(b1_d[i], [128, 1]) for i in range(L)]
            b2b = [ld(b2b_d[i], [128, H]) for i in range(L)]
            gbb = [ld(gbb_d[i], [128, H]) for i in range(L)]
            bbb = [ld(bbb_d[i], [128, H]) for i in range(L)]
            abb = [ld(abb_d[i], [128, H]) for i in range(L)]
            l1W = [ld(l1W_d[k], [128, 128]) for k in range(4)]
            l1b = ld(l1b_d, [128, 1])
            l2W = ld(l2W_d, [128, H])
            l2b = ld(l2b_d, [H, 1])
            oW = ld(oW_d, [H, 1])
            iotab = ld(iotab_d, [128, 32], BF16)
            ident = ld(ident_d, [128, 128])
            epsb = pp.tile([128, 1], F32)
            nc.vector.memset(epsb[:], EPS_BN)

            # persistent state
            ledger = pp.tile([128, NBLK, (L + 1) * H], F32)
            usc = pp.tile([128, NBLK, H], F32)       # h0n during conv, u in GEN
            ab = pp.tile([128, NBLK, 2 * H], BF16)

            assert NIDX_A == NIDX_B
            nidx_subreg = nc.gpsimd.to_reg(NIDX_A // GATHER_SPLIT)

            def edge_phase(tag, drain_fn, mid_hook=None):
                """Shared edge machinery over ag_out. drain_fn(blk, psum_tile);
                mid_hook() fires after the half-way block's drain."""
                table_dram, nch = ag_out, 2 * H
                table_dtype = BF16
                dstA_t, dstB_t = dstA16, dstB16
                sdt = BF16
                with (
                    tc.tile_pool(name=f"ep_{tag}", bufs=1) as ep,
                    tc.tile_pool(name=f"epp_{tag}", bufs=2, space="PSUM") as epp,
                    tc.tile_pool(name=f"mpp_{tag}", bufs=2, space="PSUM") as mpp,
                ):
                    for sc in range(NSC):
                        ia = idxA_sb[:, sc * (NIDX_A // 16) : (sc + 1) * (NIDX_A // 16)]
                        ib = idxB_sb[:, sc * (NIDX_B // 16) : (sc + 1) * (NIDX_B // 16)]
                        ga = ep.tile([128, NT_CH_A, nch], table_dtype, tag="ga", bufs=3)
                        gb = ep.tile([128, NT_CH_B, nch], table_dtype, tag="gb", bufs=3)
                        if EDGE_GATHER:
                            GS = GATHER_SPLIT
                            tpc = NT_CH_A // GS      # tiles per sub-call
                            nn = tpc * 128
                            for k in range(GS):
                                nc.gpsimd.dma_gather(
                                    ga[:, k * tpc : (k + 1) * tpc, :],
                                    table_dram[0 : s.SPLIT, :],
                                    ia[:, k * (nn // 16) : (k + 1) * (nn // 16)],
                                    nn, nidx_subreg, nch,
                                    queue_num=0,
                                )
                                nc.gpsimd.dma_gather(
                                    gb[:, k * tpc : (k + 1) * tpc, :],
                                    table_dram[s.SPLIT : 2 * s.SPLIT, :],
                                    ib[:, k * (nn // 16) : (k + 1) * (nn // 16)],
                                    nn, nidx_subreg, nch,
                                    queue_num=1,
                                )
                        else:
                            nc.vector.memset(ga[:], 0.25)
                            nc.vector.memset(gb[:], 0.25)
                        sa = ep.tile([128, NT_CH_A, 32], sdt, tag="sa", bufs=2)
                        iot = iotab
                        nc.vector.tensor_tensor(
                            out=sa[:],
                            in0=dstA_t[:, sc * NT_CH_A : (sc + 1) * NT_CH_A]
                            .unsqueeze(2).broadcast_to([128, NT_CH_A, 32]),
                            in1=iot[:].unsqueeze(1).broadcast_to([128, NT_CH_A, 32]),
                            op=ALU.is_equal,
                        )
                        sb = ep.tile([128, NT_CH_B, 32], sdt, tag="sb", bufs=2)
                        nc.vector.tensor_tensor(
                            out=sb[:],
                            in0=dstB_t[:, sc * NT_CH_B : (sc + 1) * NT_CH_B]
                            .unsqueeze(2).broadcast_to([128, NT_CH_B, 32]),
                            in1=iot[:].unsqueeze(1).broadcast_to([128, NT_CH_B, 32]),
                            op=ALU.is_equal,
                        )
                        for bl in range(CHUNK_BINS // 4):
                            blk = sc * (CHUNK_BINS // 4) + bl
                            ps = epp.tile([128, nch], F32, tag="eps", space="PSUM")
                            if not EDGE_MM:
                                nc.vector.memset(ps[:], 0.0)
                                drain_fn(blk, ps, ep, mpp)
                                continue
                            for j in range(4):
                                lbin = bl * 4 + j       # bin within superchunk
                                for t in range(TA):
                                    nc.tensor.matmul(
                                        out=ps[32 * j : 32 * j + 32, :],
                                        lhsT=sa[:, lbin * TA + t, :],
                                        rhs=ga[:, lbin * TA + t, :],
                                        start=(t == 0),
                                        stop=False,
                                        tile_position=(0, 32 * j),
                                    )
                                for t in range(TB):
                                    nc.tensor.matmul(
                                        out=ps[32 * j : 32 * j + 32, :],
                                        lhsT=sb[:, lbin * TB + t, :],
                                        rhs=gb[:, lbin * TB + t, :],
                                        start=False,
                                        stop=(t == TB - 1),
                                        tile_position=(0, 32 * j),
                                    )
                            drain_fn(blk, ps, ep, mpp)
                            if mid_hook is not None and blk == HALFB - 1:
                                mid_hook()

            CH = (L + 1) * H
            lbf = pp.tile([128, NBLK, CH], BF16)     # bf16 pooling copy

            # ---- lagged batched node phase ----
            # Drains only collect LN stats per block (DVE); every G8 blocks a
            # batched tail computes LN -> PReLU -> u (usc) and the softmax
            # table rows [exp(t*v), v*exp(t*v)] into ab (bf16) as 3D group
            # ops. Batching keeps Sqrt/Exp activation-table reloads to 2 per
            # group instead of 2 per block, and it all overlaps the gathers.
            G8 = 8
            mvall = pp.tile([128, NBLK, 2], F32)

            def node_stats(i, blk, ep):
                h = ledger[:, blk, i * H : (i + 1) * H]
                st = ep.tile([128, 6], F32, tag="nst", bufs=3)
                nc.vector.bn_stats(out=st[:], in_=h)
                nc.vector.bn_aggr(out=mvall[:, blk, :], in_=st[:])

            def node_tail(i, lo, hi, ep):
                W = hi - lo
                rstd = ep.tile([128, G8], F32, tag="nrstd", bufs=2)
                nc.scalar.activation(
                    out=rstd[:, 0:W], in_=mvall[:, lo:hi, 1], func=AF.Sqrt,
                    bias=epsb[:], scale=1.0,
                )
                nc.vector.reciprocal(out=rstd[:, 0:W], in_=rstd[:, 0:W])
                nmr = ep.tile([128, G8], F32, tag="nnmr", bufs=2)
                nc.vector.tensor_tensor(
                    out=nmr[:, 0:W], in0=mvall[:, lo:hi, 0], in1=rstd[:, 0:W],
                    op=ALU.mult,
                )
                nc.vector.tensor_scalar(
                    out=nmr[:, 0:W], in0=nmr[:, 0:W], scalar1=-1.0,
                    scalar2=None, op0=ALU.mult,
                )
                hsl = ledger[:, lo:hi, i * H : (i + 1) * H]
                u = usc[:, lo:hi, :]
                rb = rstd[:, 0:W].unsqueeze(2).broadcast_to([128, W, H])
                nb_ = nmr[:, 0:W].unsqueeze(2).broadcast_to([128, W, H])
                gbig = gbb[i][:].unsqueeze(1).broadcast_to([128, W, H])
                bbig = bbb[i][:].unsqueeze(1).broadcast_to([128, W, H])
                abig = abb[i][:].unsqueeze(1).broadcast_to([128, W, H])
                nc.vector.tensor_tensor(out=u, in0=hsl, in1=rb, op=ALU.mult)
                nc.vector.tensor_tensor(out=u, in0=u, in1=nb_, op=ALU.add)
                nc.vector.tensor_tensor(out=u, in0=u, in1=gbig, op=ALU.mult)
                nc.vector.tensor_tensor(out=u, in0=u, in1=bbig, op=ALU.add)
                r = ep.tile([128, G8, H], F32, tag="nrl", bufs=2)
                nc.vector.tensor_scalar(
                    out=r[:, 0:W, :], in0=u, scalar1=0.0, scalar2=None,
                    op0=ALU.max,
                )
                mneg = ep.tile([128, G8, H], F32, tag="nmg", bufs=2)
                nc.vector.tensor_tensor(
                    out=mneg[:, 0:W, :], in0=u, in1=r[:, 0:W, :], op=ALU.subtract
                )
                nc.vector.tensor_tensor(
                    out=mneg[:, 0:W, :], in0=mneg[:, 0:W, :], in1=abig, op=ALU.mult
                )
                nc.vector.tensor_add(u, r[:, 0:W, :], mneg[:, 0:W, :])
                vb = ep.tile([128, G8, H], F32, tag="nvb", bufs=2)
                nc.vector.tensor_scalar(
                    out=vb[:, 0:W, :], in0=u, scalar1=0.0, scalar2=EPS_MSG,
                    op0=ALU.max, op1=ALU.add,
                )
                Ab = ep.tile([128, G8, H], F32, tag="nAb", bufs=2)
                nc.scalar.activation(
                    out=Ab[:, 0:W, :], in_=vb[:, 0:W, :], func=AF.Exp,
                    scale=float(f["gen_t"][i]),
                )
                nc.vector.tensor_copy(out=ab[:, lo:hi, 0:H], in_=Ab[:, 0:W, :])
                nc.vector.tensor_tensor(
                    out=ab[:, lo:hi, H : 2 * H], in0=vb[:, 0:W, :],
                    in1=Ab[:, 0:W, :], op=ALU.mult,
                )

            def node_step(i, blk, ep):
                node_stats(i, blk, ep)
                if (blk + 1) % G8 == 0 or blk == NBLK - 1:
                    node_tail(i, blk - (blk % G8), blk + 1, ep)

            def pool_prep(blk, ep):
                if blk == 0:
                    nc.vector.memset(ledger[0:1, 0, 0:CH], -3.0e38)
                nc.vector.tensor_copy(out=lbf[:, blk, :], in_=ledger[:, blk, :])

            # ================= conv =================
            with (
                tc.tile_pool(name="cvp", bufs=4, space="PSUM") as cvp,
                tc.tile_pool(name="cvs", bufs=1) as cvs,
            ):
                # one DMA for all of x; the cvs pool closes before the edge
                # pools open, so the tile doesn't stack with gather buffers
                xt_all = cvs.tile([F_IN, NSLOT], F32, tag="xt_all")
                nc.sync.dma_start(out=xt_all[:], in_=xT_d[:, :])
                nc.vector.memset(ab[:, :, H : 2 * H], 0.0)
                for blk in range(NBLK):
                    h0ps = cvp.tile([128, H], F32, space="PSUM")
                    nc.tensor.matmul(
                        out=h0ps[:],
                        lhsT=xt_all[:, blk * 128 : (blk + 1) * 128],
                        rhs=Wc[:],
                        start=True, stop=True,
                    )
                    nc.vector.tensor_scalar(
                        out=usc[:, blk, :], in0=h0ps[:],
                        scalar1=dinv[:, blk : blk + 1], scalar2=None,
                        op0=ALU.mult,
                    )
                nc.vector.tensor_copy(out=ab[:, :, 0:H], in_=usc[:])

            def conv_drain(blk, ps, ep, mpp):
                t1 = ep.tile([128, H], F32, tag="cd", bufs=3)
                nc.vector.tensor_add(t1[:], ps[:, 0:H], usc[:, blk, :])
                nc.vector.tensor_scalar(
                    out=t1[:], in0=t1[:],
                    scalar1=dinv[:, blk : blk + 1], scalar2=None, op0=ALU.mult,
                )
                nc.vector.tensor_add(t1[:], t1[:], btotb[:])
                nc.vector.tensor_scalar(
                    out=ledger[:, blk, 0:H], in0=t1[:],
                    scalar1=0.0, scalar2=mask[:, blk : blk + 1],
                    op0=ALU.max, op1=ALU.mult,
                )
                node_step(0, blk, ep)

            # ================= edge phases =================
            # conv table (= [h0n | 0] bf16) is in ab; each GEN layer's drain
            # writes ledger[i+1] and fuses the next node phase (or pool prep).
            def make_gen_drain(i):
                def gen_drain(blk, ps, ep, mpp):
                    sden = ep.tile([128, H], F32, tag="sden", bufs=3)
                    nc.vector.tensor_scalar(
                        out=sden[:], in0=ps[:, 0:H], scalar1=1e-30, scalar2=None,
                        op0=ALU.add,
                    )
                    nc.vector.reciprocal(out=sden[:], in_=sden[:])
                    agg = ep.tile([128, H], F32, tag="agg", bufs=3)
                    nc.vector.tensor_tensor(
                        out=agg[:], in0=ps[:, H : 2 * H], in1=sden[:], op=ALU.mult
                    )
                    nc.vector.tensor_add(agg[:], agg[:], usc[:, blk, :])
                    tps = mpp.tile([H, 128], F32, tag="tps", space="PSUM")
                    nc.tensor.transpose(out=tps[:], in_=agg[:], identity=ident[:])
                    aggT = ep.tile([H, 128], F32, tag="aggT", bufs=3)
                    nc.vector.tensor_copy(out=aggT[:], in_=tps[:])
                    z1ps = mpp.tile([128, 128], F32, tag="z1", space="PSUM")
                    nc.tensor.matmul(
                        out=z1ps[:], lhsT=W1[i][:], rhs=aggT[:], start=True, stop=True
                    )
                    z1r = ep.tile([128, 128], F32, tag="z1r", bufs=3)
                    nc.scalar.activation(
                        out=z1r[:], in_=z1ps[:], func=AF.Relu, bias=b1[i][:], scale=1.0
                    )
                    z2ps = mpp.tile([128, H], F32, tag="z2", space="PSUM")
                    nc.tensor.matmul(
                        out=z2ps[:], lhsT=z1r[:], rhs=W2[i][:], start=True, stop=True
                    )
                    t2 = ep.tile([128, H], F32, tag="t2", bufs=3)
                    nc.vector.tensor_add(t2[:], z2ps[:], b2b[i][:])
                    nc.vector.tensor_add(t2[:], t2[:], ledger[:, blk, i * H : (i + 1) * H])
                    nc.vector.tensor_scalar(
                        out=ledger[:, blk, (i + 1) * H : (i + 2) * H], in0=t2[:],
                        scalar1=mask[:, blk : blk + 1], scalar2=None, op0=ALU.mult,
                    )
                    if i + 1 < L:
                        node_step(i + 1, blk, ep)
                    else:
                        pool_prep(blk, ep)
                return gen_drain

            HALFB = -(-(NBLK // 2) // G8) * G8   # G8-aligned: ab half complete

            def ab_dma(lo, hi):
                nc.sync.dma_start(
                    out=ag_in.ap().rearrange("(b p) c -> p b c", p=128)[:, lo:hi, :],
                    in_=ab[:, lo:hi, :],
                )

            drains = [conv_drain] + [make_gen_drain(i) for i in range(L)]
            for phase in range(L + 1):
                if phase == 0:
                    ab_dma(0, NBLK)       # conv table written during conv compute
                else:
                    ab_dma(HALFB, NBLK)   # first half went out mid prior phase
                allgather(ag_in, ag_out)
                mid = (lambda: ab_dma(0, HALFB)) if phase < L else None
                edge_phase(f"e{phase}", drains[phase], mid_hook=mid)

            # ================= pooling + head =================
            if PHASES < 3:
                dbg = nc.dram_tensor("dbg", [128, NBLK, (L + 1) * H], F32,
                                     kind="ExternalOutput")
                nc.sync.dma_start(out=dbg[:, :, :], in_=ledger[:])
            from contextlib import ExitStack as _ES
            with _ES() as _pool_ctx:
              if PHASES >= 3:
                qp = _pool_ctx.enter_context(tc.tile_pool(name="pool", bufs=1))
                qpp = _pool_ctx.enter_context(
                    tc.tile_pool(name="poolps", bufs=2, space="PSUM")
                )
                GPER = 2                          # graphs per pool-gather call
                PGS = GPER * SG                   # idxs per sub-call
                gnidx_reg = nc.gpsimd.to_reg(PGS)
                pooled = qp.tile([128, 4, GPC], F32)
                for which, gidx_d, pscale_d in (
                    (0, gidxm_d, pminv_d),
                    (1, gidxx_d, pmax_d),
                ):
                    gi = qp.tile([128, GPC * SG // 16], I16, tag="gi", bufs=2)
                    nc.sync.dma_start(out=gi[:], in_=gidx_d[:, :])
                    psc = qp.tile([128, GPC], F32, tag="psc", bufs=2)
                    nc.sync.dma_start(out=psc[:], in_=pscale_d[:, :])
                    redw = qp.tile([128, 2, GPC], F32, tag="redw", bufs=2)
                    red_op = nc.vector.reduce_sum if which == 0 else nc.vector.reduce_max
                    for k in range(GPC // GPER):
                        grid = qp.tile([128, 2, PGS], BF16, tag="grid", bufs=2)
                        nc.gpsimd.dma_gather(
                            grid[:, :, :],
                            lbf[:].rearrange("p b c -> p (b c)"),
                            gi[:, k * (PGS // 16) : (k + 1) * (PGS // 16)],
                            PGS, gnidx_reg, CH,
                            transpose=True,
                            sbuf_tokens_per_rank=128,
                            sbuf_free_dim_per_rank=CH * 2,
                            queue_num=k % 2,
                        )
                        for half in range(2):
                            red_op(
                                out=redw[:, half, k * GPER : (k + 1) * GPER],
                                in_=grid[:, half, :].rearrange(
                                    "p (m t) -> p m t", t=SG
                                ),
                                axis=mybir.AxisListType.X,
                            )
                    for half in range(2):
                        nc.vector.tensor_tensor(
                            out=pooled[:, which * 2 + half, :], in0=redw[:, half, :],
                            in1=psc[:], op=ALU.mult,
                        )
                nc.sync.dma_start(
                    out=pool_in.ap().rearrange("k p g -> p k g"), in_=pooled[:]
                )
                if MOCK_COLLECTIVES:
                    nc.sync.dma_start(
                        out=pool_out[0, :, :, :], in_=pool_in[:, :, :]
                    )
                else:
                    nc.gpsimd.collective_compute(
                        "AllGather", ALU.bypass, replica_groups=RG,
                        ins=[pool_in[:, :, :]], outs=[pool_out[:, :, :, :]],
                    )
                # head
                hps = qpp.tile([128, s.G], F32, tag="hps", space="PSUM")
                pk = []
                for k in range(4):
                    t = qp.tile([128, NCORES, GPC], F32, tag=f"pk{k}")
                    nc.sync.dma_start(
                        out=t[:], in_=pool_out[:, k, :, :].rearrange("r p g -> p r g")
                    )
                    pk.append(t)
                for k in range(4):
                    nc.tensor.matmul(
                        out=hps[:], lhsT=l1W[k][:],
                        rhs=pk[k][:].rearrange("p r g -> p (r g)"),
                        start=(k == 0), stop=(k == 3),
                    )
                hz1 = qp.tile([128, s.G], F32)
                nc.scalar.activation(
                    out=hz1[:], in_=hps[:], func=AF.Relu, bias=l1b[:], scale=1.0
                )
                h2ps = qpp.tile([H, s.G], F32, tag="h2ps", space="PSUM")
                nc.tensor.matmul(out=h2ps[:], lhsT=l2W[:], rhs=hz1[:], start=True, stop=True)
                hz2 = qp.tile([H, s.G], F32)
                nc.scalar.activation(
                    out=hz2[:], in_=h2ps[:], func=AF.Relu, bias=l2b[:], scale=1.0
                )
                ops = qpp.tile([1, s.G], F32, tag="ops", space="PSUM")
                nc.tensor.matmul(out=ops[:], lhsT=oW[:], rhs=hz2[:], start=True, stop=True)
                osb = qp.tile([1, s.G], F32)
                nc.vector.tensor_scalar(
                    out=osb[:], in0=ops[:], scalar1=float(f["out_b"][0]),
                    scalar2=None, op0=ALU.add,
                )
                nc.sync.dma_start(out=out_d.ap().rearrange("g one -> one g"), in_=osb[:])

    nc.compile()
    return nc


def _insert_library_loads(nc):
    import bass_rust as _bass_rust
    from concourse.library_config import all_libraries, standard

    inst_type_to_lib_mask = {}
    for lib in all_libraries:
        for inst_type in lib.instructions:
            inst_type_to_lib_mask[inst_type] = inst_type_to_lib_mask.get(
                inst_type, 0
            ) | (1 << lib.index)
    _bass_rust.insert_library_loads(
        nc, inst_type_to_lib_mask, len(all_libraries), standard.index
    )


# ---------------------------------------------------------------- wait split
def split_waits(nc, max_waits: int = 1) -> int:
    nsplit = 0
    for fn in nc.m.functions:
        for bb in fn.blocks:
            new_insts = []
            for ins in bb.instructions:
                si = ins.sync_info
                if si is not None and si.on_wait and len(si.on_wait) > max_waits:
                    waits = list(si.on_wait)
                    spill, keep = waits[:-max_waits], waits[-max_waits:]
                    for k, w in enumerate(spill):
                        nop = mybir.InstNoOp(
                            name=f"{ins.name}-wsplit{k}",
                            engine=ins.engine,
                            bass_nofuse=True,
                            sync_info=mybir.SyncInfo(on_wait=[w], on_update=[]),
                        )
                        new_insts.append(nop)
                        nc.register_instruction(nop, overwrite=True)
                        nsplit += 1
                    si.on_wait = keep
                new_insts.append(ins)
            if len(new_insts) != len(bb.instructions):
                bb.instructions[:] = new_insts
    return nsplit


# ---------------------------------------------------------------- entry
def kernel(**inputs) -> np.ndarray:
    x = np.asarray(inputs["x"], np.float32)
    ei = np.asarray(inputs["edge_index"], np.int64)
    bi = np.asarray(inputs["batch_idx"], np.int64)
    G = 256
    s = build_schedule(ei, bi, G)
    f = fold_weights(inputs)
    maps = build_inmaps(s, x)
    nc = build_nc(s, f)
    res = run_bass_kernel_spmd(nc, maps, core_ids=list(range(NCORES)))
    raw = np.asarray(res.results[0]["out"], np.float32)
    out = np.empty_like(raw)
    out[s.graph_order] = raw              # undo the graph->core permutation
    return out

